# revision 30
# baseline (speedup 1.0000x reference)
"""Multi-head self-attention (B=4, S=2048, D=1024, H=16) on 8 TRN2 NeuronCores.

Sharding: core c handles batch b = c // 2 and head-group g = c % 2
(8 heads, 512 hidden columns). Per core:
  - Q^T, K^T projections (d-major layout), V projection (token-major).
  - Attention computed transposed: S^T[k, q] = K_h @ Q_h^T per 128-key
    block, exp on the scalar engine (softmax max-subtraction skipped --
    logits are ~N(0,1), exp cannot overflow). Causal structure exploited
    at column granularity: fully masked blocks skipped, and for diagonal
    blocks only the visible column suffix is computed (scores N, exp,
    mask multiply, and AV N are all restricted to [c0, 512)).
  - O^T = V_aug^T @ P^T with a ones column appended to V so the softmax
    denominator drops out of the same matmul; normalize by reciprocal.
  - Per query group: the two cores of a batch AllGather their bf16
    attention-output halves (O_attn^T, d-major), then each core computes
    the FULL-row output projection for its own 512 output columns:
    out[:, g*512:(g+1)*512] = O_attn @ wo[:, gcols] + bo[gcols].
    No reduction needed -- output columns are disjoint.
Host reassembles: out[b, :, g*512:(g+1)*512] = core (2*b + g).

Queue discipline (engine queues are FIFO): the scalar engine runs the
exp activations and startup loads only (all loads precede the first exp
in program order); gpsimd handles attention-phase plumbing (it is blocked
~20us at start by the collectives-init op); sync carries bulk DMA.
"""

import sys

for _p in ("/opt/trn_rl_repo",):
    if _p not in sys.path:
        sys.path.insert(0, _p)

from contextlib import ExitStack

import ml_dtypes
import numpy as np

import concourse.bass as bass
from concourse import bacc
import concourse.mybir as mybir
import concourse.tile as tile
from concourse.bass_utils import run_bass_kernel_spmd

F32 = mybir.dt.float32
BF16 = mybir.dt.bfloat16
MM_DT = BF16
AF = mybir.ActivationFunctionType
ALU = mybir.AluOpType

B, S, D, H, DEPTH = 4, 2048, 1024, 16, 64
HG = H // 2          # heads per core = 8
GD = HG * DEPTH      # local hidden width = 512
QG = 512             # query-group width (matmul N)
KB = 128             # key-block height (matmul M)
NQG = S // QG        # 4
NKB = S // KB        # 16
NCORES = 8
NPAIR = 4            # head pairs per core

LAST_EXEC_NS = None
LAST_RESULTS = None

QG_ORDER = (0, 1, 2, 3)   # small qgs first (overlap projections), big last


def _mask_schedule(mask2d):
    """Classify each (query-group, key-block) against the actual mask.

    Returns (sched, windows): sched[qg] is a list of (kb, c0, wlen, widx);
    a block is skipped entirely when fully masked. Columns [0, c0) of the
    block are fully masked (skipped), columns [c0+wlen, 512) fully visible;
    the mask varies only inside [c0, c0+wlen), covered by 0/1 window
    `widx` ([128, wlen], transposed to [k, q]). wlen == 0 => fully visible.
    """
    wins = {}
    warr = []
    sched = []
    for qg in range(NQG):
        blocks = []
        for kb in range(NKB):
            blk = mask2d[qg * QG:(qg + 1) * QG, kb * KB:(kb + 1) * KB]  # [q, k]
            col_masked = blk.all(axis=1)      # fully-masked query columns
            col_full = (blk == 0.0).all(axis=1)  # fully-visible query columns
            if col_masked.all():
                continue
            # c0: leading fully-masked columns
            c0 = 0
            while c0 < QG and col_masked[c0]:
                c0 += 1
            # c1: start of fully-visible suffix
            c1 = QG
            while c1 > c0 and col_full[c1 - 1]:
                c1 -= 1
            wlen = c1 - c0
            if wlen == 0:
                blocks.append((kb, c0, 0, None))
                continue
            assert set(np.unique(blk)) <= {0.0, 1.0}, "non-binary mask"
            w = (1.0 - np.ascontiguousarray(blk[c0:c1, :].T)).astype(np.float32)
            key = w.tobytes()
            if key not in wins:
                wins[key] = len(warr)
                warr.append(w)
            blocks.append((kb, c0, wlen, wins[key]))
        sched.append(blocks)
    assert len({w.shape for w in warr} | set()) <= 1, "window shapes differ"
    windows = np.stack(warr) if warr else None
    assert windows is None or len(warr) <= 16, "too many unique mask windows"
    return sched, windows


def _build(sched, windows):
    nc = bacc.Bacc(target_bir_lowering=False, trn_type="TRN2")

    # bulk inputs pre-swizzled host-side into SBUF tile layout so each
    # partition's data is one contiguous run (fewer, bigger DMA descriptors)
    xq = nc.dram_tensor("xq_r", [2, 128, 4 * S], BF16, kind="ExternalInput")
    xk = nc.dram_tensor("xk_r", [2, 128, 4 * S], BF16, kind="ExternalInput")
    xv = nc.dram_tensor("xv_r", [2, 128, 4 * S], BF16, kind="ExternalInput")
    wq_d = nc.dram_tensor("wq_r", [128, 8 * GD], BF16, kind="ExternalInput")
    wk_d = nc.dram_tensor("wk_r", [128, 8 * GD], BF16, kind="ExternalInput")
    wv_d = nc.dram_tensor("wv_r", [128, 8 * GD], BF16, kind="ExternalInput")
    wo_d = nc.dram_tensor("wo_r", [128, 8 * GD], BF16, kind="ExternalInput")
    bq_d = nc.dram_tensor("bq_g", [GD], F32, kind="ExternalInput")
    bk_d = nc.dram_tensor("bk_g", [GD], F32, kind="ExternalInput")
    bv_d = nc.dram_tensor("bv_g", [GD], F32, kind="ExternalInput")
    bo_d = nc.dram_tensor("bo_g", [GD], F32, kind="ExternalInput")
    out_d = nc.dram_tensor("out", [S, GD], F32, kind="ExternalOutput")

    ones_d = nc.inline_tensor(np.ones((128, HG), dtype=ml_dtypes.bfloat16), "ones_col")
    mwin_d = None
    nwin = 0
    wlen0 = 0
    if windows is not None:
        nwin = windows.shape[0]
        wlen0 = windows.shape[2]
        mwin_d = nc.inline_tensor(windows.astype(ml_dtypes.bfloat16), "mwin")

    qg_last = QG_ORDER[-1]

    with tile.TileContext(nc) as tc, ExitStack() as ctx:
        persist = ctx.enter_context(tc.tile_pool(name="persist", bufs=1))
        dram = ctx.enter_context(tc.tile_pool(name="dram", bufs=1, space="DRAM"))

        # per-qg DRAM staging for the O_attn^T exchange (bf16, d-major)
        oout = [dram.tile([GD, QG], BF16, tag=f"oout{qg}", name=f"oout{qg}")
                for qg in range(NQG)]
        oall = [dram.tile([2, GD, QG], BF16, tag=f"oall{qg}", name=f"oall{qg}")
                for qg in range(NQG) if qg != qg_last]
        oall.insert(qg_last, None)
        # last qg: per-pair chunked exchange for a shorter tail
        oallc = dram.tile([NPAIR, 2, KB, QG], BF16, tag="oallc", name="oallc")

        # ---- persistent SBUF tensors -------------------------------------
        qt = [persist.tile([128, S], MM_DT, tag=f"qt{i}", name=f"qt{i}")
              for i in range(NPAIR)]
        kt = [persist.tile([128, S], MM_DT, tag=f"kt{i}", name=f"kt{i}")
              for i in range(NPAIR)]
        # V with a ones column appended per head: [tokens, head, 64 V + 1]
        vt = [persist.tile([128, HG, DEPTH + 1], MM_DT, tag=f"vt{t}", name=f"vt{t}")
              for t in range(NKB)]
        mw = [persist.tile([KB, wlen0 or 1], MM_DT, tag=f"mw{w}", name=f"mw{w}")
              for w in range(nwin)]
        bo_bc = persist.tile([128, GD], F32, tag="bo_bc")
        bv_bc = persist.tile([128, GD], F32, tag="bv_bc")
        bqt = [persist.tile([128, 1], F32, tag=f"bq{m}", name=f"bq{m}") for m in range(4)]
        bkt = [persist.tile([128, 1], F32, tag=f"bk{m}", name=f"bk{m}") for m in range(4)]

        for m in range(4):
            nc.sync.dma_start(out=bqt[m], in_=bq_d[m * 128:(m + 1) * 128])
            nc.scalar.dma_start(out=bkt[m], in_=bk_d[m * 128:(m + 1) * 128])

        def load_consts():
            """Attention-phase constants: emitted after the bulk loads so the
            slow SWDGE small-DMA setup never starves the projection feed."""
            for w in range(nwin):
                nc.gpsimd.dma_start(out=mw[w], in_=mwin_d[w, :, :])
            nc.gpsimd.dma_start(
                out=bo_bc, in_=bass.AP(tensor=bo_d, offset=0, ap=[[0, 128], [1, GD]]))
            nc.gpsimd.dma_start(
                out=bv_bc, in_=bass.AP(tensor=bv_d, offset=0, ap=[[0, 128], [1, GD]]))
            for t in range(NKB):
                nc.gpsimd.dma_start(out=vt[t][:, :, DEPTH:DEPTH + 1], in_=ones_d[:, :])

        pps = ctx.enter_context(tc.tile_pool(name="pps", bufs=2, space="PSUM"))
        pacc = ctx.enter_context(tc.tile_pool(name="pacc", bufs=3, space="PSUM"))
        ppo = ctx.enter_context(tc.tile_pool(name="ppo", bufs=1, space="PSUM"))

        xw_pool = ctx.enter_context(tc.tile_pool(name="xw", bufs=4))
        w_pool = ctx.enter_context(tc.tile_pool(name="wp", bufs=2))
        pt_pool = ctx.enter_context(tc.tile_pool(name="ptp", bufs=14))
        ot_pool = ctx.enter_context(tc.tile_pool(name="otp", bufs=2))
        nrm_pool = ctx.enter_context(tc.tile_pool(name="nrm", bufs=2))
        osb_pool = ctx.enter_context(tc.tile_pool(name="osb", bufs=1))

        LOAD_ENGS = (nc.sync, nc.scalar, nc.gpsimd)   # 3-queue load rotation

        def load_x(xd):
            """Load pre-swizzled x as two xw tiles [128, 4, S], one contiguous
            2MB DMA each (16KB per partition per descriptor)."""
            tiles = []
            for h in range(2):
                xt = xw_pool.tile([128, 4, S], MM_DT, tag="xt", name=f"xt{h}")
                LOAD_ENGS[h % 2].dma_start(out=xt[:, :, :], in_=xd[h, :, :])
                tiles.append(xt)
            return tiles

        def load_w(wd):
            wt = w_pool.tile([128, 8, GD], MM_DT, tag="wt", name="wt")
            nc.gpsimd.dma_start(out=wt[:, :, :], in_=wd[:, :])
            return wt

        def proj_T(xtiles, wt, bias_tiles, dst, m_list):
            """dst[m][:, :] = (x @ w + b)^T  (d on partitions), m-chunk major."""
            for m in m_list:
                for tg2 in range(2):
                    ps = pps.tile([128, 2 * QG], F32, tag="ps", name="ps")
                    for tgh in range(2):
                        tg = tg2 * 2 + tgh
                        for kk in range(8):
                            nc.tensor.matmul(
                                ps[:, tgh * QG:(tgh + 1) * QG],
                                wt[:, kk, m * 128:(m + 1) * 128],
                                xtiles[kk // 4][:, kk % 4, tg * QG:(tg + 1) * QG],
                                start=(kk == 0),
                                stop=(kk == 7),
                            )
                    for tgh in range(2):
                        tg = tg2 * 2 + tgh
                        nc.vector.tensor_scalar_add(
                            dst[m][:, tg * QG:(tg + 1) * QG],
                            ps[:, tgh * QG:(tgh + 1) * QG],
                            bias_tiles[m][:, :],
                        )

        def vproj_chain(xtiles, wvt, t):
            """One token-tile V projection chain -> vt[t] (single PSUM bank)."""
            ps = ppo.tile([128, GD], F32, tag="po", name="po")
            for kk in range(8):
                nc.tensor.matmul(
                    ps[:, :],
                    xtiles[kk // 4][:, kk % 4, t * 128:(t + 1) * 128],
                    wvt[:, kk, :],
                    start=(kk == 0),
                    stop=(kk == 7),
                )
            nc.vector.tensor_tensor(
                vt[t][:, :, 0:DEPTH],
                ps[:, :].rearrange("p (h d) -> p h d", h=HG),
                bv_bc[:, :].rearrange("p (h d) -> p h d", h=HG),
                ALU.add,
            )

        # ---- attention helpers ------------------------------------------
        def scores_exp_block(qg, i, blk):
            """One block: restricted scores + exp + mask. Returns (kb, c0, pt)."""
            kb, c0, wlen, wix = blk
            kbc = slice(kb * KB, (kb + 1) * KB)
            n = QG - c0
            sps = pps.tile([128, 2 * QG], F32, tag="ps", name="ps")
            for p, off in ((0, 0), (1, 64)):
                nc.tensor.matmul(
                    sps[:, p * QG + c0:(p + 1) * QG],
                    kt[i][off:off + 64, kbc],
                    qt[i][off:off + 64, qg * QG + c0:(qg + 1) * QG],
                    start=True,
                    stop=True,
                )
            pt = pt_pool.tile([KB, 2 * QG], MM_DT, tag="pt", name="pt")
            if c0 == 0:
                nc.scalar.activation(pt[:, :], sps[:, :], AF.Exp, scale=0.125)
            else:
                nc.scalar.activation(
                    pt.rearrange("k (t q) -> k t q", t=2)[:, :, c0:],
                    sps.rearrange("k (t q) -> k t q", t=2)[:, :, c0:],
                    AF.Exp, scale=0.125)
            if wix is not None:
                m_ap = mw[wix][:, 0:wlen]
                rep = bass.AP(
                    tensor=m_ap.tensor,
                    offset=m_ap.offset,
                    ap=[list(m_ap.ap[0]), [0, 2], list(m_ap.ap[1])],
                )
                nc.vector.tensor_tensor(
                    pt.rearrange("k (t q) -> k t q", t=2)[:, :, c0:c0 + wlen],
                    pt.rearrange("k (t q) -> k t q", t=2)[:, :, c0:c0 + wlen],
                    rep,
                    ALU.mult,
                )
            return (kb, c0, pt)

        ots = {}

        def av_mms(qg, i, accs, pts, lo, hi, nb):
            """AV chain segment for blocks [lo, hi) of pair (qg, i)."""
            for bi in range(lo, hi):
                kb, c0, pt = pts[bi]
                for p in range(2):
                    nc.tensor.matmul(
                        accs[p][0:DEPTH + 1, c0:],
                        vt[kb][:, 2 * i + p, :],
                        pt[:, p * QG + c0:(p + 1) * QG],
                        start=(bi == 0),
                        stop=(bi == nb - 1),
                    )

        def norm(qg, i, accs):
            """Normalize (divide by the ones-column row) -> ots[qg][i]."""
            ot = ots[qg]
            for p, acc in enumerate(accs):
                o_un = nrm_pool.tile([DEPTH + 1, QG], F32, tag="o_un", name="o_un")
                nc.vector.tensor_copy(o_un[64:65, :], acc[64:65, :])
                rc0 = nrm_pool.tile([1, QG], F32, tag="rc0", name="rc0")
                nc.sync.dma_start(out=rc0[:, :], in_=o_un[64:65, :])
                rcr = nrm_pool.tile([1, QG], F32, tag="rcr", name="rcr")
                nc.vector.reciprocal_approx_fast(rcr[:, :], rc0[:, :])
                rb = nrm_pool.tile([64, QG], F32, tag="rb", name="rb")
                nc.gpsimd.partition_broadcast(rb[:, :], rcr[:, :])
                if p == 0:
                    nc.vector.tensor_tensor(
                        ot[i][0:64, :], acc[0:64, :], rb[:, :], ALU.mult)
                else:
                    tmp = nrm_pool.tile([64, QG], MM_DT, tag="tmp", name="tmp")
                    nc.vector.tensor_tensor(
                        tmp[:, :], acc[0:64, :], rb[:, :], ALU.mult)
                    nc.sync.dma_start(out=ot[i][64:128, :], in_=tmp[:, :])
            nc.sync.dma_start(
                out=oout[qg][i * 128:(i + 1) * 128, :], in_=ot[i][:, :])
            if qg == qg_last:
                nc.gpsimd.collective_compute(
                    "AllGather",
                    ALU.bypass,
                    replica_groups=[[0, 1], [2, 3], [4, 5], [6, 7]],
                    ins=[oout[qg][i * 128:(i + 1) * 128, :]],
                    outs=[oallc[i, :, :, :]],
                )

        def new_accs(p):
            return [pacc.tile([128, QG], F32, tag="acc", name=f"acc{p_}")
                    for p_ in range(2)]

        def av_norm(qg, i, pts):
            accs = new_accs(i)
            av_mms(qg, i, accs, pts, 0, len(pts), len(pts))
            norm(qg, i, accs)

        def new_ot(qg):
            ots[qg] = [ot_pool.tile([128, QG], MM_DT, tag=f"ot{i}", name=f"ot{i}")
                       for i in range(NPAIR)]

        def fire_ag(qg):
            nc.gpsimd.collective_compute(
                "AllGather",
                ALU.bypass,
                replica_groups=[[0, 1], [2, 3], [4, 5], [6, 7]],
                ins=[oout[qg][:, :]],
                outs=[oall[qg][:, :, :]],
            )

        def oproj_load(qg):
            """Both halves of O_attn^T for qg, one consolidated DMA per rank."""
            full = [ot_pool.tile([128, 4, QG], MM_DT, tag=f"fl{r}", name=f"fl{r}")
                    for r in range(2)]
            for r in range(2):
                if qg != qg_last:
                    t = oall[qg]
                    src = bass.AP(tensor=t.tensor, offset=t.offset + r * GD * QG,
                                  ap=[[QG, 128], [128 * QG, 4], [1, QG]])
                else:
                    t = oallc
                    src = bass.AP(tensor=t.tensor, offset=t.offset + r * KB * QG,
                                  ap=[[QG, 128], [2 * KB * QG, 4], [1, QG]])
                nc.sync.dma_start(out=full[r][:, :, :], in_=src)
            return full

        def oproj_ts(qg, full, ts):
            po = ppo.tile([128, QG], F32, tag="po", name="po")
            for j in range(8):
                nc.tensor.matmul(
                    po[:, :],
                    full[j // 4][:, j % 4, ts * 128:(ts + 1) * 128],
                    wosb[:, j, :],
                    start=(j == 0),
                    stop=(j == 7),
                )
            osb = osb_pool.tile([128, GD], F32, tag="osb", name="osb")
            nc.vector.tensor_tensor(osb[:, :], po[:, :], bo_bc[:, :], ALU.add)
            nc.sync.dma_start(
                out=out_d[qg * QG + ts * 128: qg * QG + (ts + 1) * 128, :],
                in_=osb[:, :])

        # ---- emission schedule ------------------------------------------
        wkt = load_w(wk_d)
        xk_tiles = load_x(xk)
        wqt = load_w(wq_d)
        xq_tiles = load_x(xq)
        wvt = load_w(wv_d)
        xv_tiles = load_x(xv)
        wosb = load_w(wo_d)
        load_consts()

        proj_T(xk_tiles, wkt, bkt, kt, range(4))

        # head phase: Q projection chunks, qg0 + qg1 attention, and the V
        # projection interleaved so the exp stream starts right after K+Qm0.
        # Emission order is pt-ring-safe for bufs=14 (an exp that needs ring
        # slot k is always preceded by the AV matmul that frees it).
        new_ot(0)
        new_ot(1)
        sc = lambda qg, i, lo, hi: [scores_exp_block(qg, i, blk)
                                    for blk in sched[qg][lo:hi]]
        vp = lambda lo, hi: [vproj_chain(xv_tiles, wvt, t) for t in range(lo, hi)]

        proj_T(xq_tiles, wqt, bqt, qt, [0])
        p00 = sc(0, 0, 0, 4)
        proj_T(xq_tiles, wqt, bqt, qt, [1])
        p01 = sc(0, 1, 0, 4)
        proj_T(xq_tiles, wqt, bqt, qt, [2])
        p02 = sc(0, 2, 0, 4)
        proj_T(xq_tiles, wqt, bqt, qt, [3])
        vp(0, 4)
        a00 = new_accs(0); av_mms(0, 0, a00, p00, 0, 4, 4); norm(0, 0, a00)
        p03 = sc(0, 3, 0, 4)
        vp(4, 8)
        a01 = new_accs(1); av_mms(0, 1, a01, p01, 0, 4, 4); norm(0, 1, a01)
        p10 = sc(1, 0, 0, 4)
        a02 = new_accs(2); av_mms(0, 2, a02, p02, 0, 4, 4); norm(0, 2, a02)
        p10 += sc(1, 0, 4, 8)
        vp(8, 16)
        a03 = new_accs(3); av_mms(0, 3, a03, p03, 0, 4, 4); norm(0, 3, a03)
        fire_ag(0)
        p11 = sc(1, 1, 0, 6)
        a10 = new_accs(0); av_mms(1, 0, a10, p10, 0, 8, 8); norm(1, 0, a10)
        p11 += sc(1, 1, 6, 8)
        a11 = new_accs(1); av_mms(1, 1, a11, p11, 0, 8, 8); norm(1, 1, a11)
        p12 = sc(1, 2, 0, 8)
        a12 = new_accs(2); av_mms(1, 2, a12, p12, 0, 8, 8); norm(1, 2, a12)
        p13 = sc(1, 3, 0, 8)
        a13 = new_accs(3); av_mms(1, 3, a13, p13, 0, 8, 8); norm(1, 3, a13)
        fire_ag(1)

        # qg2 with qg0's output projection interleaved
        new_ot(2)
        full0 = oproj_load(0)
        for i in range(NPAIR):
            pts = [scores_exp_block(2, i, blk) for blk in sched[2]]
            av_norm(2, i, pts)
            oproj_ts(0, full0, i)
        fire_ag(2)

        # qg3 (chunked AG fired inside norm) with qg1+qg2 O-proj; pairs have
        # 16 blocks > 14 pt bufs, so the AV chain is emitted in two segments
        new_ot(3)
        full1 = oproj_load(1)
        full2 = oproj_load(2)
        for i in range(NPAIR):
            pts = sc(3, i, 0, 13)
            accs = new_accs(i)
            av_mms(3, i, accs, pts, 0, 2, 16)
            pts += sc(3, i, 13, 16)
            av_mms(3, i, accs, pts, 2, 16, 16)
            norm(3, i, accs)
            oproj_ts(1, full1, i)
            if i >= 1:
                oproj_ts(2, full2, i - 1)
        oproj_ts(2, full2, 3)
        full3 = oproj_load(3)
        for ts in range(4):
            oproj_ts(3, full3, ts)

    nc.finalize()
    return nc


_CACHED = {}


def _get_nc(mask2d):
    key = mask2d.tobytes()
    if key not in _CACHED:
        _CACHED[key] = _build(*_mask_schedule(mask2d))
    return _CACHED[key]


def kernel(v, k, q, mask, wq, bq, wk, bk, wv, bv, wo, bo, _trace=False):
    global LAST_EXEC_NS, LAST_RESULTS
    f = lambda a: np.asarray(a, dtype=np.float32)
    v, k, q = f(v), f(k), f(q)
    wq, wk, wv, wo = f(wq), f(wk), f(wv), f(wo)
    bq, bk, bv, bo = f(bq), f(bk), f(bv), f(bo)
    mask2d = f(mask).reshape(S, S)

    nc = _get_nc(mask2d)

    bf = lambda a: np.ascontiguousarray(a).astype(ml_dtypes.bfloat16)

    def swz_x(xt):
        # x^T [D, S] -> [2 tiles, 128 partitions, 4*S] (kk-major per partition)
        return bf(xt.reshape(2, 4, 128, S).transpose(0, 2, 1, 3)
                  .reshape(2, 128, 4 * S))

    def swz_w(w):
        # w [D, GD] -> [128, 8*GD] (kk-major per partition)
        return bf(w.reshape(8, 128, GD).transpose(1, 0, 2).reshape(128, 8 * GD))

    in_maps = []
    for c in range(NCORES):
        b, g = c // 2, c % 2
        cols = slice(g * GD, (g + 1) * GD)
        in_maps.append({
            "xq_r": swz_x(q[b].T),
            "xk_r": swz_x(k[b].T),
            "xv_r": swz_x(v[b].T),
            "wq_r": swz_w(wq[:, cols]),
            "wk_r": swz_w(wk[:, cols]),
            "wv_r": swz_w(wv[:, cols]),
            "wo_r": swz_w(wo[:, cols]),
            "bq_g": np.ascontiguousarray(bq[cols]),
            "bk_g": np.ascontiguousarray(bk[cols]),
            "bv_g": np.ascontiguousarray(bv[cols]),
            "bo_g": np.ascontiguousarray(bo[cols]),
        })

    res = run_bass_kernel_spmd(
        nc, in_maps, core_ids=list(range(NCORES)), trace=_trace
    )
    LAST_EXEC_NS = res.exec_time_ns
    LAST_RESULTS = res

    out = np.empty((B, S, D), dtype=np.float32)
    for c in range(NCORES):
        b, g = c // 2, c % 2
        o = res.results[c]["out"]  # [S, GD]
        out[b, :, g * GD:(g + 1) * GD] = o
    return out
